# revision 8
# baseline (speedup 1.0000x reference)
"""Per-row cosine similarity: out[b, n] = <a[b,n,:], b[b,n,:]> / (||a[b,n,:]|| * ||b[b,n,:]||).

Inputs a, b: [32, 2048, 1024] f32. Output: [32, 2048] f32.

Strategy: batch-shard across 8 NeuronCores (4 batches = 8192 rows per core).
Each core streams its 64 MiB through SBUF on the single SP HWDGE queue in
[128 rows, Tx1024] tiles (4 KiB-per-partition interleaved descriptors -
both a per-partition-contiguous layout and a 2-queue split measurably hurt:
the former thrashes SBUF banks against engine reads, the latter makes the
SDMA engines alternate rings per packet). Steps taper (4,...,4,2,1,1 tiles)
so only ~2 us of compute remains after the final byte lands; mid-stream
behavior is identical to the uniform-step version.

Per 128-row tile, three fused elementwise+row-sum ops:
  - dot(a,b): DVE tensor_tensor via scalar_tensor_tensor (one pass, accum)
  - sum(a^2): ACT activation(Square, accum_out=...)
  - sum(b^2): alternates DVE/ACT per tile to balance engine load (final
    tile forced to DVE, with its b-load issued first, to keep the tail off
    the slower ACT chain)
Both engines stay under the DMA roofline (~400 GB/s/core effective), so the
kernel is memory-bound end to end. The epilogue computes
dot/sqrt(max(sa,eps)*max(sb,eps)) with a Newton-refined sqrt, transposes
[128, 64] -> [64, 128] on TensorE, and writes the 32 KiB result with one
contiguous DMA. (A chunked mid-stream epilogue was tried and rejected: the
extra in-order DVE<->ACT ping-pong destabilizes the WAR-gated DMA pipeline
into a ~12 us/step limit cycle.)
"""

import numpy as np

import concourse.bass as bass
import concourse.bacc as bacc
import concourse.mybir as mybir
import concourse.tile as tile
from concourse.bass_utils import run_bass_kernel_spmd
from concourse.masks import make_identity

N_CORES = 8
B, N, D = 32, 2048, 1024
ROWS_PER_CORE = (B // N_CORES) * N  # 8192
P = 128
N_TILES = ROWS_PER_CORE // P  # 64
STEP_SIZES = [4] * 15 + [2, 1, 1]  # tiles per step; sum == N_TILES
IO_BUFS = 4
EPS = 1e-12

_cache: dict = {}
last_results = None  # BassKernelResults of the most recent run (for test harness)


def _build() -> bass.Bass:
    if "nc" in _cache:
        return _cache["nc"]

    f32 = mybir.dt.float32
    mult = mybir.AluOpType.mult

    nc = bacc.Bacc(trn_type="TRN2")
    a_d = nc.dram_tensor("a", [ROWS_PER_CORE, D], f32, kind="ExternalInput")
    b_d = nc.dram_tensor("b", [ROWS_PER_CORE, D], f32, kind="ExternalInput")
    o_d = nc.dram_tensor("o", [ROWS_PER_CORE], f32, kind="ExternalOutput")

    # Row r = t*P + p: tile-major, interleaved per-partition descriptors.
    a_v = a_d.rearrange("(t p) d -> p t d", p=P)
    b_v = b_d.rearrange("(t p) d -> p t d", p=P)

    with (
        tile.TileContext(nc) as tc,
        tc.tile_pool(name="io", bufs=IO_BUFS) as io,
        tc.tile_pool(name="scr", bufs=2) as scr,
        tc.tile_pool(name="aux", bufs=1) as aux,
        tc.tile_pool(name="ps", bufs=1, space="PSUM") as ps_pool,
    ):
        # Per-row statistics, one column per 128-row tile.
        dot = aux.tile([P, N_TILES], f32)
        sa = aux.tile([P, N_TILES], f32)
        sbE = aux.tile([P, N_TILES // 2], f32)  # sum(b^2), even tiles (DVE)
        sbO = aux.tile([P, N_TILES // 2], f32)  # sum(b^2), odd tiles (ACT)

        # The fused reduce ops must write their full-size elementwise result
        # somewhere; rotating scratch tiles keep consecutive ops independent.
        # (InstTensorTensorReduce and stride-0 broadcast outputs both crash the
        # exec unit on this runtime, so: scalar_tensor_tensor + real scratch.)
        def dve_dot(in0, in1, acc):
            dve_scr = scr.tile([P, D], f32, tag="dve_scr")
            nc.vector.scalar_tensor_tensor(
                out=dve_scr,
                in0=in0,
                scalar=1.0,
                in1=in1,
                op0=mult,
                op1=mult,
                accum_out=acc,
            )

        def act_sumsq(in0, acc):
            act_scr = scr.tile([P, D], f32, tag="act_scr")
            nc.scalar.activation(
                out=act_scr,
                in_=in0,
                func=mybir.ActivationFunctionType.Square,
                accum_out=acc,
            )

        t0 = 0
        for T in STEP_SIZES:
            a_sb = io.tile([P, T, D], f32, tag="a_sb")
            b_sb = io.tile([P, T, D], f32, tag="b_sb")
            if T == 1:
                # Tail steps: land b first so sum(b^2) overlaps the a-load.
                nc.sync.dma_start(out=b_sb, in_=b_v[:, t0 : t0 + T, :])
                nc.sync.dma_start(out=a_sb, in_=a_v[:, t0 : t0 + T, :])
            else:
                nc.sync.dma_start(out=a_sb, in_=a_v[:, t0 : t0 + T, :])
                nc.sync.dma_start(out=b_sb, in_=b_v[:, t0 : t0 + T, :])
            for j in range(T):
                t = t0 + j
                aj = a_sb[:, j, :]
                bj = b_sb[:, j, :]
                if t == N_TILES - 1:
                    # Final tile: sum(b^2) first (its data lands first) and on
                    # DVE, keeping the post-stream tail off the slower ACT.
                    dve_dot(bj, bj, sbO[:, t // 2 : t // 2 + 1])
                    dve_dot(aj, bj, dot[:, t : t + 1])
                    act_sumsq(aj, sa[:, t : t + 1])
                    continue
                dve_dot(aj, bj, dot[:, t : t + 1])
                act_sumsq(aj, sa[:, t : t + 1])
                if t % 2 == 0:
                    dve_dot(bj, bj, sbE[:, t // 2 : t // 2 + 1])
                else:
                    act_sumsq(bj, sbO[:, t // 2 : t // 2 + 1])
            t0 += T

        # Epilogue: out = dot / sqrt(max(sa, EPS) * max(sb, EPS)), per row.
        H = N_TILES // 2
        dotv = dot.rearrange("p (i par) -> p par i", par=2)
        sav = sa.rearrange("p (i par) -> p par i", par=2)
        outT = aux.tile([P, N_TILES], f32)
        outTv = outT.rearrange("p (i par) -> p par i", par=2)
        mA = aux.tile([P, H], f32)
        mB = aux.tile([P, H], f32)
        d2 = aux.tile([P, H], f32)
        sq = aux.tile([P, H], f32)
        rc = aux.tile([P, H], f32)
        t1 = aux.tile([P, H], f32)
        for par, sbH in ((0, sbE), (1, sbO)):
            nc.vector.tensor_scalar_max(mA, sav[:, par, :], EPS)
            nc.vector.tensor_scalar_max(mB, sbH, EPS)
            nc.vector.tensor_mul(d2, mA, mB)
            # sqrt with one Newton step: s1 = 0.5*(s + d2/s); ACT sqrt alone
            # has a loose ULP budget.
            nc.scalar.sqrt(sq, d2)
            nc.vector.reciprocal(rc, sq)
            nc.vector.tensor_mul(t1, d2, rc)
            nc.vector.tensor_add(t1, t1, sq)
            nc.vector.tensor_scalar_mul(t1, t1, 0.5)
            nc.vector.reciprocal(rc, t1)
            nc.vector.tensor_mul(outTv[:, par, :], dotv[:, par, :], rc)

        # outT[p, t] holds the result for row t*128+p. Transpose on TensorE so
        # the store is one contiguous DMA.
        ident = aux.tile([P, P], f32)
        make_identity(nc, ident)
        ps_t = ps_pool.tile([N_TILES, P], f32)
        nc.tensor.transpose(ps_t, outT, ident)
        outF = aux.tile([N_TILES, P], f32)
        nc.scalar.copy(outF, ps_t)
        nc.sync.dma_start(out=o_d.rearrange("(t p) -> t p", p=P), in_=outF)

    nc.finalize()
    _cache["nc"] = nc
    return nc


def kernel(a: np.ndarray, b: np.ndarray, trace: bool = False, **run_kwargs) -> np.ndarray:
    global last_results
    nc = _build()
    a = np.ascontiguousarray(np.asarray(a, dtype=np.float32)).reshape(
        N_CORES, ROWS_PER_CORE, D
    )
    b = np.ascontiguousarray(np.asarray(b, dtype=np.float32)).reshape(
        N_CORES, ROWS_PER_CORE, D
    )
    in_maps = [{"a": a[k], "b": b[k]} for k in range(N_CORES)]
    res = run_bass_kernel_spmd(
        nc, in_maps, core_ids=list(range(N_CORES)), trace=trace, **run_kwargs
    )
    last_results = res
    out = np.stack([res.results[k]["o"] for k in range(N_CORES)])
    return out.reshape(B, N).astype(np.float32, copy=False)
